# revision 6
# baseline (speedup 1.0000x reference)
"""Trainium2 (8 NeuronCores) kernel for batched multi-head causal attention.

Problem: q,k,v [4, 16, 2048, 64] f32, attention_mask [4, 1, 2048] (all ones).
Reference: softmax((q@k^T + causal_mask) * 1/sqrt(64)) @ v.

Sharding: pure data/head parallelism. B*H = 64 heads, 8 per core.

v2 design notes (ScalarE/exp is the bottleneck engine; everything else is
arranged around keeping it saturated with pure exp work):
  - Scores are computed transposed, S^T[s, l] (s on partitions), in f32 PSUM
    "chunks" of [128, 1024] (2 banks).  Diagonal s-tiles emit only their
    visible column range, and pieces are PACKED back-to-back inside chunks
    (each piece kept inside a 512-col bank slot) so every exp instruction
    covers a dense span: 20 ACTIVATEs per head, zero wasted columns.
  - exp on ScalarE with the 1/sqrt(D) scale folded in; output pt is f32 in
    SBUF, consumed by the PV matmul as float32r (full-rate f32 matmul), so V
    needs no bf16 cast and is loaded straight from HBM.
  - q/k f32->bf16 casts run on GpSimd (ScalarE must stay exp-only).
  - Causal masking post-exp via gpsimd affine_select restricted to the 128
    partial-visibility columns of each diagonal tile (4 per l-tile).
  - QK^T matmuls have K=64: heads A/B are packed on partitions 0:64/64:128 of
    QT/KT, and their matmuls are emitted interleaved so the PE runs them
    concurrently as row-tiles (tile_position auto-derives from base partition).
  - Softmax denominator from an appended ones-column on V (PV stationary is
    [128, 65]); output is unnormalized, transposed back (PE), scaled by the
    reciprocal row-sum (DVE).
"""

import numpy as np
from contextlib import ExitStack

# problem shape (hardcoded; kernel.py must be self-contained)
B, H, S, D = 4, 16, 2048, 64
NCORES = 8
NH = (B * H) // NCORES   # 8 heads per core
ST = 128                 # s-tile (key) rows per matmul
NST = S // ST            # 16 s-tiles
LT = 512                 # l-tile (query) columns per PV psum bank
NLT = S // LT            # 4 l-tiles
CW = 1024                # score-chunk width (f32 cols; 2 psum banks)
SCALE = 1.0 / float(np.sqrt(D))

_CACHE = {}


def chunk_layout(lt):
    """Pack the visible score pieces of l-tile `lt` into [128, CW] chunks.

    Returns a list of chunks; each chunk is a list of (t, off, pos, w):
    s-tile t's scores S^T[128t:128t+128, l0+off : l0+LT] land at chunk
    columns [pos, pos+w).  Pieces never cross a 512-col psum bank boundary.
    """
    n_s = 4 * lt + 4
    l0 = LT * lt
    chunks, cur, pos = [], [], 0
    for t in range(n_s):
        c0 = t * ST - l0
        off = c0 if c0 > 0 else 0
        w = LT - off
        if pos % 512 + w > 512:
            pos = (pos // 512 + 1) * 512
        if pos + w > CW:
            chunks.append(cur)
            cur, pos = [], 0
        cur.append((t, off, pos, w))
        pos += w
    if cur:
        chunks.append(cur)
    return chunks


def _build_nc(reps=1, bodies=1):
    import concourse.bacc as bacc
    import concourse.bass as bass
    import concourse.mybir as mybir
    import concourse.tile as tile
    from concourse.masks import make_identity

    F32 = mybir.dt.float32
    BF16 = mybir.dt.bfloat16

    nc = bacc.Bacc("TRN2", target_bir_lowering=False, debug=False, num_devices=NCORES)

    q_d = nc.dram_tensor("q", [NH, S, D], F32, kind="ExternalInput")
    k_d = nc.dram_tensor("k", [NH, S, D], F32, kind="ExternalInput")
    v_d = nc.dram_tensor("v", [NH, S, D], F32, kind="ExternalInput")
    o_d = nc.dram_tensor("out", [NH, S, D], F32, kind="ExternalOutput")

    with tile.TileContext(nc) as tc, ExitStack() as ctx:
        const = ctx.enter_context(tc.tile_pool(name="const", bufs=1))
        nat = ctx.enter_context(tc.tile_pool(name="nat", bufs=2))
        natb = ctx.enter_context(tc.tile_pool(name="natb", bufs=2))
        natv = ctx.enter_context(tc.tile_pool(name="natv", bufs=4))
        qkt = ctx.enter_context(tc.tile_pool(name="qkt", bufs=3))
        pts = ctx.enter_context(tc.tile_pool(name="pts", bufs=3))
        ovs = ctx.enter_context(tc.tile_pool(name="ovs", bufs=2))
        rts = ctx.enter_context(tc.tile_pool(name="rts", bufs=2))
        osb = ctx.enter_context(tc.tile_pool(name="osb", bufs=4))
        psc = ctx.enter_context(tc.tile_pool(name="psc", bufs=3, space="PSUM"))
        ppv = ctx.enter_context(tc.tile_pool(name="ppv", bufs=2, space="PSUM"))

        identb = const.tile([128, 128], BF16, tag="identb")
        make_identity(nc, identb[:])

        import contextlib

        _eng = mybir.EngineType
        loop = (
            tc.For_i(0, reps, 1,
                     hint_engines=(_eng.PE, _eng.DVE, _eng.Activation, _eng.Pool, _eng.SP))
            if reps > 1
            else contextlib.nullcontext()
        )
        with loop:
            for _body_i in range(bodies):
                _emit_body(nc, tc, mybir,
                           const, nat, natb, natv, qkt, pts, ovs, rts, osb,
                           psc, ppv, identb, q_d, k_d, v_d, o_d)

    nc.compile()
    return nc


def _emit_body(nc, tc, mybir,
               const, nat, natb, natv, qkt, pts, ovs, rts, osb,
               psc, ppv, identb, q_d, k_d, v_d, o_d):
    F32 = mybir.dt.float32
    F32R = mybir.dt.float32r
    BF16 = mybir.dt.bfloat16
    EXP = mybir.ActivationFunctionType.Exp

    for pair in range(NH // 2):
        hA, hB = 2 * pair, 2 * pair + 1

        # ---- load q/k with heads A|B packed along d; cast on GpSimd -------
        def load_pair(src, tag):
            raw = nat.tile([128, NST, 2 * D], F32, tag=tag + "f")
            for i, h in enumerate((hA, hB)):
                nc.sync.dma_start(
                    out=raw[:, :, i * D : (i + 1) * D],
                    in_=src.ap()[h].rearrange("(t p) d -> p t d", p=128),
                )
            t = natb.tile([128, NST, 2 * D], BF16, tag=tag)
            nc.gpsimd.tensor_copy(t[:], raw[:])
            return t

        qn = load_pair(q_d, "qn")
        kn = load_pair(k_d, "kn")

        # ---- transpose q/k into [64, 2048] per head, packed A|B ----------
        QT = qkt.tile([128, S], BF16, tag="QT")
        KT = qkt.tile([128, S], BF16, tag="KT")
        for dst, srct in ((QT, qn), (KT, kn)):
            done = 0
            while done < NST:
                n = min(8, NST - done)
                stg = psc.tile([128, CW], BF16, tag="sc")
                for j in range(n):
                    nc.tensor.transpose(
                        stg[:, 128 * j : 128 * (j + 1)], srct[:, done + j, :], identb[:]
                    )
                nc.vector.tensor_copy(
                    dst[:, ST * done : ST * (done + n)], stg[:, 0 : 128 * n]
                )
                done += n

        # ---- V natural with ones column, bf16 (cast on GpSimd) -----------
        def load_v(h):
            raw = natv.tile([128, NST, D], F32, tag="vn")
            nc.sync.dma_start(
                out=raw[:],
                in_=v_d.ap()[h].rearrange("(t p) d -> p t d", p=128),
            )
            t = natv.tile([128, NST, D + 1], BF16, tag="vr")
            nc.gpsimd.tensor_copy(t[:, :, 0:D], raw[:])
            nc.gpsimd.memset(t[:, :, D : D + 1], 1.0)
            return t

        vA = load_v(hA)
        vB = load_v(hB)
        vts = {hA: vA, hB: vB}

        # ---- attention, heads A/B interleaved at chunk granularity -------
        outsb_A = osb.tile([128, NST, D], F32, tag="outsb")
        outsb_B = osb.tile([128, NST, D], F32, tag="outsb")
        outsb_all = {hA: outsb_A, hB: outsb_B}
        for lt in range(NLT):
            l0 = lt * LT
            n_s = 4 * lt + 4
            layout = chunk_layout(lt)

            pvt_A = ppv.tile([D + 1, LT], F32, tag="pv")
            pvt_B = ppv.tile([D + 1, LT], F32, tag="pv")
            pvts = {hA: pvt_A, hB: pvt_B}

            for chunk in layout:
                used = chunk[-1][2] + chunk[-1][3]
                scs, ptts = {}, {}
                for h in (hA, hB):
                    scs[h] = psc.tile([128, CW], F32, tag="sc", name="sc")
                    ptts[h] = pts.tile([128, CW], BF16, tag="pt", name="pt")
                # scores: A/B matmuls interleaved -> PE row-tile concurrency
                for (t, off, pos, w) in chunk:
                    for h, rb in ((hA, 0), (hB, 64)):
                        nc.tensor.matmul(
                            scs[h][:, pos : pos + w],
                            lhsT=KT[rb : rb + 64, t * ST : (t + 1) * ST],
                            rhs=QT[rb : rb + 64, l0 + off : l0 + LT],
                            start=True,
                            stop=True,
                        )
                for h in (hA, hB):
                    sc, pt = scs[h], ptts[h]
                    nc.scalar.activation(pt[:, 0:used], sc[:, 0:used], EXP, scale=SCALE)
                    # causal masking on the 128 partial-visibility cols of
                    # each diagonal tile (c0 >= 0): keep where col >= row
                    for (t, off, pos, w) in chunk:
                        if t * ST - l0 >= 0:
                            nc.gpsimd.affine_select(
                                out=pt[:, pos : pos + 128],
                                in_=pt[:, pos : pos + 128],
                                compare_op=mybir.AluOpType.is_ge,
                                fill=0.0,
                                base=0,
                                channel_multiplier=-1,
                                pattern=[[1, 128]],
                            )
                    vt = vts[h]
                    pvt = pvts[h]
                    for (t, off, pos, w) in chunk:
                        nc.tensor.matmul(
                            pvt[:, off:LT],
                            lhsT=vt[:, t, :],
                            rhs=pt[:, pos : pos + w],
                            start=(t == 0),
                            stop=(t == n_s - 1),
                        )

            # ---- epilogue: transpose back, normalize by row-sum ----------
            for h in (hA, hB):
                outsb = outsb_all[h]
                pvt = pvts[h]
                ovt = ovs.tile([D + 1, LT], BF16, tag="ov")
                nc.vector.tensor_copy(ovt[:], pvt[:])
                ost = ppv.tile([128, 4 * (D + 4)], BF16, tag="pv")
                for j in range(4):
                    nc.tensor.transpose(
                        ost[:, (D + 4) * j : (D + 4) * j + D + 1],
                        ovt[:, 128 * j : 128 * (j + 1)],
                        identb[0 : D + 1, 0 : D + 1],
                    )
                osr = ost[:].rearrange("p (j c) -> p j c", c=D + 4)
                rt = rts.tile([128, 4], F32, tag="rt")
                nc.vector.reciprocal(rt[:], osr[:, :, D])
                nc.vector.tensor_mul(
                    outsb[:, 4 * lt : 4 * lt + 4, :],
                    osr[:, :, 0:D],
                    rt[:].unsqueeze(2).to_broadcast((128, 4, D)),
                )
        for h in (hA, hB):
            nc.sync.dma_start(
                out=o_d.ap()[h].rearrange("(c p) d -> p c d", p=128),
                in_=outsb_all[h][:],
            )


def get_nc(reps=1, bodies=1):
    key = (reps, bodies)
    if key not in _CACHE:
        _CACHE[key] = _build_nc(reps, bodies)
    return _CACHE[key]


def make_in_maps(q, k, v):
    q = np.ascontiguousarray(np.asarray(q, dtype=np.float32).reshape(B * H, S, D))
    k = np.ascontiguousarray(np.asarray(k, dtype=np.float32).reshape(B * H, S, D))
    v = np.ascontiguousarray(np.asarray(v, dtype=np.float32).reshape(B * H, S, D))
    maps = []
    for c in range(NCORES):
        sl = slice(c * NH, (c + 1) * NH)
        maps.append(
            {
                "q": np.ascontiguousarray(q[sl]),
                "k": np.ascontiguousarray(k[sl]),
                "v": np.ascontiguousarray(v[sl]),
            }
        )
    return maps


def kernel(q, k, v, attention_mask=None, **_ignored):
    """Full inputs in, full output out. attention_mask is all-ones by
    construction in this problem and drops out of the math."""
    from concourse.bass_utils import run_bass_kernel_spmd

    nc = get_nc()
    res = run_bass_kernel_spmd(nc, make_in_maps(q, k, v), core_ids=list(range(NCORES)))
    out = np.concatenate([res.results[c]["out"] for c in range(NCORES)], axis=0)
    return out.reshape(B, H, S, D).astype(np.float32)


# revision 7
# speedup vs baseline: 1.2708x; 1.2708x over previous
"""Trainium2 (8 NeuronCores) kernel for batched multi-head causal attention.

Problem: q,k,v [4, 16, 2048, 64] f32, attention_mask [4, 1, 2048] (all ones).
Reference: softmax((q@k^T + causal_mask) * 1/sqrt(64)) @ v.

Sharding: pure data/head parallelism. B*H = 64 heads, 8 per core.

Design notes (ScalarE/exp is the bottleneck engine; everything else is
arranged around keeping it saturated with pure exp work):
  - Scores are computed transposed, S^T[s, l] (s on partitions), in f32 PSUM
    "chunks" of [128, 1024] (2 banks).  Diagonal s-tiles emit only their
    visible column range, and pieces are PACKED back-to-back inside chunks
    (each piece kept inside a 512-col bank slot) so every exp instruction
    covers a dense span: 20 ACTIVATEs per head, zero wasted columns.
  - exp on ScalarE with the 1/sqrt(D) scale folded in; ScalarE does nothing
    else.  q/k/v f32->bf16 casts run on DVE.
  - The input stage (DMA, cast, PE transpose of q/k) for pair p+1 is emitted
    in the middle of pair p's compute, so pair boundaries don't stall the
    exp pipeline on the DMA->cast->transpose->copy chain.
  - Causal masking post-exp via gpsimd affine_select restricted to the 128
    partial-visibility columns of each diagonal tile (4 per l-tile).
  - QK^T matmuls have K=64: heads A/B are packed on partitions 0:64/64:128 of
    QT/KT, and their matmuls are emitted interleaved so the PE runs them
    concurrently as row-tiles (tile_position auto-derives from base partition).
  - Softmax denominator from an appended ones-column on V (PV stationary is
    [128, 65]); output is unnormalized, transposed back (PE), scaled by the
    reciprocal row-sum (DVE).
"""

import numpy as np
from contextlib import ExitStack

# problem shape (hardcoded; kernel.py must be self-contained)
B, H, S, D = 4, 16, 2048, 64
NCORES = 8
NH = (B * H) // NCORES   # 8 heads per core
ST = 128                 # s-tile (key) rows per matmul
NST = S // ST            # 16 s-tiles
LT = 512                 # l-tile (query) columns per PV psum bank
NLT = S // LT            # 4 l-tiles
CW = 1024                # score-chunk width (f32 cols; 2 psum banks)
SCALE = 1.0 / float(np.sqrt(D))

_CACHE = {}


def chunk_layout(lt):
    """Pack the visible score pieces of l-tile `lt` into [128, CW] chunks.

    Returns a list of chunks; each chunk is a list of (t, off, pos, w):
    s-tile t's scores S^T[128t:128t+128, l0+off : l0+LT] land at chunk
    columns [pos, pos+w).  Pieces never cross a 512-col psum bank boundary.
    """
    n_s = 4 * lt + 4
    l0 = LT * lt
    chunks, cur, pos = [], [], 0
    for t in range(n_s):
        c0 = t * ST - l0
        off = c0 if c0 > 0 else 0
        w = LT - off
        if pos % 512 + w > 512:
            pos = (pos // 512 + 1) * 512
        if pos + w > CW:
            chunks.append(cur)
            cur, pos = [], 0
        cur.append((t, off, pos, w))
        pos += w
    if cur:
        chunks.append(cur)
    return chunks


def _build_nc(reps=1, bodies=1):
    import concourse.bacc as bacc
    import concourse.mybir as mybir
    import concourse.tile as tile
    from concourse.masks import make_identity

    F32 = mybir.dt.float32
    BF16 = mybir.dt.bfloat16

    nc = bacc.Bacc("TRN2", target_bir_lowering=False, debug=False, num_devices=NCORES)

    q_d = nc.dram_tensor("q", [NH, S, D], F32, kind="ExternalInput")
    k_d = nc.dram_tensor("k", [NH, S, D], F32, kind="ExternalInput")
    v_d = nc.dram_tensor("v", [NH, S, D], F32, kind="ExternalInput")
    o_d = nc.dram_tensor("out", [NH, S, D], F32, kind="ExternalOutput")

    with tile.TileContext(nc) as tc, ExitStack() as ctx:
        const = ctx.enter_context(tc.tile_pool(name="const", bufs=1))
        nat = ctx.enter_context(tc.tile_pool(name="nat", bufs=2))
        natb = ctx.enter_context(tc.tile_pool(name="natb", bufs=2))
        natvr = ctx.enter_context(tc.tile_pool(name="natvr", bufs=2))
        natv = ctx.enter_context(tc.tile_pool(name="natv", bufs=4))
        qkt = ctx.enter_context(tc.tile_pool(name="qkt", bufs=4))
        pts = ctx.enter_context(tc.tile_pool(name="pts", bufs=3))
        ovs = ctx.enter_context(tc.tile_pool(name="ovs", bufs=2))
        rts = ctx.enter_context(tc.tile_pool(name="rts", bufs=2))
        osb = ctx.enter_context(tc.tile_pool(name="osb", bufs=4))
        psc = ctx.enter_context(tc.tile_pool(name="psc", bufs=3, space="PSUM"))
        ppv = ctx.enter_context(tc.tile_pool(name="ppv", bufs=2, space="PSUM"))

        identb = const.tile([128, 128], BF16, tag="identb")
        make_identity(nc, identb[:])

        import contextlib

        _eng = mybir.EngineType
        loop = (
            tc.For_i(0, reps, 1,
                     hint_engines=(_eng.PE, _eng.DVE, _eng.Activation, _eng.Pool, _eng.SP))
            if reps > 1
            else contextlib.nullcontext()
        )
        with loop:
            for _body_i in range(bodies):
                _emit_body(nc, tc, mybir,
                           nat, natb, natvr, natv, qkt, pts, ovs, rts, osb,
                           psc, ppv, identb, q_d, k_d, v_d, o_d)

    nc.compile()
    return nc


def _emit_body(nc, tc, mybir,
               nat, natb, natvr, natv, qkt, pts, ovs, rts, osb,
               psc, ppv, identb, q_d, k_d, v_d, o_d):
    F32 = mybir.dt.float32
    BF16 = mybir.dt.bfloat16
    EXP = mybir.ActivationFunctionType.Exp

    def input_stage(pair):
        """DMA q/k/v, cast to bf16 (DVE), transpose q/k on PE into
        QT/KT [128(A|B packed on d), S]."""
        hA, hB = 2 * pair, 2 * pair + 1

        def load_pair(src, tag):
            raw = nat.tile([128, NST, 2 * D], F32, tag=tag + "f", name="raw")
            for i, h in enumerate((hA, hB)):
                nc.sync.dma_start(
                    out=raw[:, :, i * D : (i + 1) * D],
                    in_=src.ap()[h].rearrange("(t p) d -> p t d", p=128),
                )
            t = natb.tile([128, NST, 2 * D], BF16, tag=tag, name="natb")
            nc.vector.tensor_copy(t[:], raw[:])
            return t

        qn = load_pair(q_d, "qn")
        kn = load_pair(k_d, "kn")

        QT = qkt.tile([128, S], BF16, tag="QT", name="QT")
        KT = qkt.tile([128, S], BF16, tag="KT", name="KT")
        for dst, srct in ((QT, qn), (KT, kn)):
            done = 0
            while done < NST:
                n = min(8, NST - done)
                stg = psc.tile([128, CW], BF16, tag="sc", name="stg")
                for j in range(n):
                    nc.tensor.transpose(
                        stg[:, 128 * j : 128 * (j + 1)], srct[:, done + j, :], identb[:]
                    )
                nc.vector.tensor_copy(
                    dst[:, ST * done : ST * (done + n)], stg[:, 0 : 128 * n]
                )
                done += n

        def load_v(h):
            raw = natvr.tile([128, NST, D], F32, tag="vn", name="vraw")
            nc.sync.dma_start(
                out=raw[:],
                in_=v_d.ap()[h].rearrange("(t p) d -> p t d", p=128),
            )
            t = natv.tile([128, NST, D + 1], BF16, tag="vr", name="vr")
            nc.vector.tensor_copy(t[:, :, 0:D], raw[:])
            nc.gpsimd.memset(t[:, :, D : D + 1], 1.0)
            return t

        return {
            "QT": QT,
            "KT": KT,
            "v": {hA: load_v(hA), hB: load_v(hB)},
        }

    def compute_lts(pair, st, lts, outsb_all):
        hA, hB = 2 * pair, 2 * pair + 1
        QT, KT, vts = st["QT"], st["KT"], st["v"]
        for lt in lts:
            l0 = lt * LT
            n_s = 4 * lt + 4
            layout = chunk_layout(lt)

            pvt_A = ppv.tile([D + 1, LT], F32, tag="pv", name="pv")
            pvt_B = ppv.tile([D + 1, LT], F32, tag="pv", name="pv")
            pvts = {hA: pvt_A, hB: pvt_B}

            for chunk in layout:
                used = chunk[-1][2] + chunk[-1][3]
                scs, ptts = {}, {}
                for h in (hA, hB):
                    scs[h] = psc.tile([128, CW], F32, tag="sc", name="sc")
                    ptts[h] = pts.tile([128, CW], BF16, tag="pt", name="pt")
                # scores: A/B matmuls interleaved -> PE row-tile concurrency
                for (t, off, pos, w) in chunk:
                    for h, rb in ((hA, 0), (hB, 64)):
                        nc.tensor.matmul(
                            scs[h][:, pos : pos + w],
                            lhsT=KT[rb : rb + 64, t * ST : (t + 1) * ST],
                            rhs=QT[rb : rb + 64, l0 + off : l0 + LT],
                            start=True,
                            stop=True,
                        )
                for h in (hA, hB):
                    sc, pt = scs[h], ptts[h]
                    nc.scalar.activation(pt[:, 0:used], sc[:, 0:used], EXP, scale=SCALE)
                    # causal masking on the 128 partial-visibility cols of
                    # each diagonal tile (c0 >= 0): keep where col >= row
                    for (t, off, pos, w) in chunk:
                        if t * ST - l0 >= 0:
                            nc.gpsimd.affine_select(
                                out=pt[:, pos : pos + 128],
                                in_=pt[:, pos : pos + 128],
                                compare_op=mybir.AluOpType.is_ge,
                                fill=0.0,
                                base=0,
                                channel_multiplier=-1,
                                pattern=[[1, 128]],
                            )
                    vt = vts[h]
                    pvt = pvts[h]
                    for (t, off, pos, w) in chunk:
                        nc.tensor.matmul(
                            pvt[:, off:LT],
                            lhsT=vt[:, t, :],
                            rhs=pt[:, pos : pos + w],
                            start=(t == 0),
                            stop=(t == n_s - 1),
                        )

            # ---- epilogue: transpose back, normalize by row-sum ----------
            for h in (hA, hB):
                outsb = outsb_all[h]
                pvt = pvts[h]
                ovt = ovs.tile([D + 1, LT], BF16, tag="ov", name="ov")
                nc.vector.tensor_copy(ovt[:], pvt[:])
                ost = ppv.tile([128, 4 * (D + 4)], BF16, tag="pv", name="ost")
                for j in range(4):
                    nc.tensor.transpose(
                        ost[:, (D + 4) * j : (D + 4) * j + D + 1],
                        ovt[:, 128 * j : 128 * (j + 1)],
                        identb[0 : D + 1, 0 : D + 1],
                    )
                osr = ost[:].rearrange("p (j c) -> p j c", c=D + 4)
                rt = rts.tile([128, 4], F32, tag="rt", name="rt")
                nc.vector.reciprocal(rt[:], osr[:, :, D])
                nc.vector.tensor_mul(
                    outsb[:, 4 * lt : 4 * lt + 4, :],
                    osr[:, :, 0:D],
                    rt[:].unsqueeze(2).to_broadcast((128, 4, D)),
                )

    # software-pipelined: pair p+1's input stage is emitted between pair p's
    # first and second halves, so its DMA/cast/transposes overlap compute.
    st = input_stage(0)
    for pair in range(NH // 2):
        hA, hB = 2 * pair, 2 * pair + 1
        outsb_A = osb.tile([128, NST, D], F32, tag="outsb", name="outsb")
        outsb_B = osb.tile([128, NST, D], F32, tag="outsb", name="outsb")
        outsb_all = {hA: outsb_A, hB: outsb_B}
        compute_lts(pair, st, [0, 1], outsb_all)
        nxt = input_stage(pair + 1) if pair + 1 < NH // 2 else None
        compute_lts(pair, st, [2, 3], outsb_all)
        for h in (hA, hB):
            nc.sync.dma_start(
                out=o_d.ap()[h].rearrange("(c p) d -> p c d", p=128),
                in_=outsb_all[h][:],
            )
        st = nxt


def get_nc(reps=1, bodies=1):
    key = (reps, bodies)
    if key not in _CACHE:
        _CACHE[key] = _build_nc(reps, bodies)
    return _CACHE[key]


def make_in_maps(q, k, v):
    q = np.ascontiguousarray(np.asarray(q, dtype=np.float32).reshape(B * H, S, D))
    k = np.ascontiguousarray(np.asarray(k, dtype=np.float32).reshape(B * H, S, D))
    v = np.ascontiguousarray(np.asarray(v, dtype=np.float32).reshape(B * H, S, D))
    maps = []
    for c in range(NCORES):
        sl = slice(c * NH, (c + 1) * NH)
        maps.append(
            {
                "q": np.ascontiguousarray(q[sl]),
                "k": np.ascontiguousarray(k[sl]),
                "v": np.ascontiguousarray(v[sl]),
            }
        )
    return maps


def kernel(q, k, v, attention_mask=None, **_ignored):
    """Full inputs in, full output out. attention_mask is all-ones by
    construction in this problem and drops out of the math."""
    from concourse.bass_utils import run_bass_kernel_spmd

    nc = get_nc()
    res = run_bass_kernel_spmd(nc, make_in_maps(q, k, v), core_ids=list(range(NCORES)))
    out = np.concatenate([res.results[c]["out"] for c in range(NCORES)], axis=0)
    return out.reshape(B, H, S, D).astype(np.float32)
